# revision 3
# baseline (speedup 1.0000x reference)
"""BoundaryLoss Trainium2 kernel (V24): fused boundary-mask-mul-reduce
custom DVE op + single-table exp/ln pipeline.

Structure per core (4 images of 512x512 + merged 16-row tails):
- host packs per image ONE fp8 tensor [128, 4, 516+512]: per 128-row
  window, 516 bytes of zero-padded 0/1 target (denormal coding: byte k
  == k*2^-9) then 512 bytes of PRESIGNED pred y=(1-2t)*x (junk lanes
  y=-240 -> bce ~ 0).
- DVE: v = t0+t2, v3 = v+t4 as u16 packed byte adds; box sum s =
  Band.T @ v3[c] + Band.T @ v[c+1] (2 fp8 matmuls/window) in PSUM.
- ACT: bce = ln(1+exp(y)) as two passes; Exp and Ln both live in the
  natural_log_exp_and_others table (get_activation_tables patched so
  the chooser must pick it) -> ONE table load, free interleaving.
  Ln runs with accum_out => per-chunk sum(bce).
- DVE custom op BND_MASK_MUL_REDUCE: out = select((s-12.5*SC)^2 < QTHR,
  spy, 0), accum_out = sum => per-chunk boundary-masked sum(bce), read
  straight from PSUM.  One op replaces the old bias-add/square/
  masked-product pipeline.
- total = sum(A_cols) + 4*sum(B_cols); mean on host.
"""

import numpy as np

import concourse.bass as bass
import concourse.bacc as bacc_mod
import concourse.tile as tile
from concourse import mybir
from concourse.bass_utils import run_bass_kernel_spmd

F32 = mybir.dt.float32
BF16 = mybir.dt.bfloat16
FP8 = mybir.dt.float8e4
U16 = mybir.dt.uint16
ALU = mybir.AluOpType
ACTF = mybir.ActivationFunctionType

B, H, W = 32, 512, 512
NCORES = 8
IMGS = B // NCORES          # 4 images per core
PAD = 2
TP = H + 2 * PAD            # 516
NWIN = 4                    # main 124-row windows per image
PKC = TP + W                # 1028 packed bytes per (partition, window)
SC = 2.0 ** -9              # denormal coding scale of the 0/1 target bytes
QTHR = 144.0 * SC * SC      # (s-12.5SC)^2 < QTHR  <=>  s in {1..24}

NA = 6                      # ln accum cols: ln0..ln2, ln3a, ln3b, lnT
NB = 9                      # bmr accum cols: pairs 0a..3b, T
NSTAT = NA + NB


def _ap3(t, off, dims):
    return bass.AP(t, off, dims)


def _patch_act_tables(arch="gen3"):
    """Only natural_log_exp_and_others may claim Exp/Ln, so the table
    chooser puts both passes in one resident table: one load, and exp/ln
    interleave with no reload or phase separation."""
    from concourse.hw_specs import get_activation_tables

    t = get_activation_tables(arch)  # functools.cache -> shared object
    A = mybir.ActivationFunctionType
    comb = t["natural_log_exp_and_others"]
    assert A.Exp in comb and A.Ln in comb
    for name, s in t.items():
        if name != "natural_log_exp_and_others":
            s.discard(A.Exp)
            s.discard(A.Ln)


def _register_bmr():
    """out = select((in0+s0)^2 < s1, in1, 0); accum_out = sum(out)."""
    from operator import add as _add

    import concourse.dve_ops as dvo
    from concourse.dve_spec import C0, C1, Spec, Src0, Src1, Zero, lower, select, sq
    from concourse.dve_spec import _has_src1
    from concourse.dve_uop import DveOpSpec

    name = "BND_MASK_MUL_REDUCE"
    if name in dvo._SUB_OPCODE_FOR_NAME:
        return next(o for o in dvo.OPS if o.name == name)

    def _ref(in0, in1, s0, s1, imm2):
        q = (in0.astype(np.float32) + s0) ** 2
        b = np.where(q < s1, in1.astype(np.float32), 0.0).astype(np.float32)
        return b, b.reshape(b.shape[0], -1).sum(axis=-1, keepdims=True)

    spec = Spec(
        body=select(sq(Src0 + C0) < C1, Src1, Zero),
        accum=_add,
        accum_init=Zero,
        reference=_ref,
    )
    row = max(dvo._SUB_OPCODE_FOR_NAME.values()) + 1
    assert row < 0x20
    dvo._SUB_OPCODE_FOR_NAME[name] = row
    shas = {}
    for ver in ("v3", "v4"):
        s = DveOpSpec(name=name, opcode=row, uops=lower(spec, ver=ver),
                      rd1_en=_has_src1(spec))
        shas[ver] = s.sha(ver)
    op = dvo.DveOp(name, spec, subdim=False, uops_sha=shas)
    dvo.OPS.append(op)
    dvo.CUSTOM_DVE_SPECS[name] = spec
    return op


def _build_nc() -> bass.Bass:
    BMR = _register_bmr()
    _patch_act_tables()
    nc = bacc_mod.Bacc(trn_type="TRN2")

    pkm = nc.dram_tensor("pkm", [IMGS, 128, NWIN, PKC], FP8, kind="ExternalInput")
    pkt = nc.dram_tensor("pkt", [128, PKC], FP8, kind="ExternalInput")
    bands = nc.dram_tensor("bands", [128, 2, 128], FP8, kind="ExternalInput")
    stats = nc.dram_tensor("stats", [128, NSTAT], F32, kind="ExternalOutput")

    with tile.TileContext(nc) as tc:
        with (
            tc.tile_pool(name="singles", bufs=1) as singles,
            tc.tile_pool(name="pkin", bufs=4) as pkin,
            tc.tile_pool(name="vp", bufs=4) as vp,
            tc.tile_pool(name="v3p", bufs=4) as v3p,
            tc.tile_pool(name="eyp", bufs=4) as eyp,
            tc.tile_pool(name="spyp", bufs=4) as spyp,
            tc.tile_pool(name="scrp", bufs=3) as scrp,
            tc.tile_pool(name="ps2", bufs=3, space="PSUM") as ps2,
            tc.tile_pool(name="ps1", bufs=1, space="PSUM") as ps1,
        ):
            # ---- input DMAs on two rings: gpsimd carries the small
            # pkt+bands (so the tail primes ACT early), sync carries the
            # four big image tensors in consumption order.
            pkt_sb = singles.tile([128, PKC], FP8)
            nc.gpsimd.dma_start(pkt_sb[:], pkt[:, :])
            band_sb = singles.tile([128, 2, 128], FP8)
            nc.gpsimd.dma_start(band_sb[:], bands[:, :, :])
            pk_sb = [None] * IMGS
            for i in range(IMGS):
                pk_sb[i] = pkin.tile([128, NWIN, PKC], FP8, tag="pk",
                                     name=f"pk{i}")
                nc.sync.dma_start(
                    pk_sb[i][:],
                    _ap3(pkm, i * 128 * NWIN * PKC,
                         [[NWIN * PKC, 128], [PKC, NWIN], [1, PKC]]),
                )

            stats_sb = singles.tile([128, NSTAT], F32)
            nc.gpsimd.memset(stats_sb[:], 0.0)

            # ---- DVE phase 1: u16 packed adds (v = t0+t2, v3 = v+t4),
            # tail first (its DMA lands first)
            dve_chain = []
            vT = singles.tile([128, TP - 2], FP8)
            dve_chain.append(nc.vector.tensor_tensor(
                vT[:].bitcast(U16),
                pkt_sb[:, 0:TP - 2].bitcast(U16),
                pkt_sb[:, 2:TP].bitcast(U16),
                op=ALU.add,
            ))
            v3T = singles.tile([128, W], FP8)
            dve_chain.append(nc.vector.tensor_tensor(
                v3T[:].bitcast(U16),
                vT[:, 0:W].bitcast(U16),
                pkt_sb[:, 4:4 + W].bitcast(U16),
                op=ALU.add,
            ))
            v_sb = [None] * IMGS
            v3_sb = [None] * IMGS
            add_insts = {}
            for i in range(IMGS):
                tpk = pk_sb[i]
                v_sb[i] = vp.tile([128, NWIN, TP - 2], FP8, tag="v", name=f"v{i}")
                a1 = nc.vector.tensor_tensor(
                    v_sb[i][:].bitcast(U16),
                    tpk[:, :, 0:TP - 2].bitcast(U16),
                    tpk[:, :, 2:TP].bitcast(U16),
                    op=ALU.add,
                )
                v3_sb[i] = v3p.tile([128, NWIN, W], FP8, tag="v3", name=f"v3{i}")
                a2 = nc.vector.tensor_tensor(
                    v3_sb[i][:].bitcast(U16),
                    v_sb[i][:, :, 0:W].bitcast(U16),
                    tpk[:, :, 4:4 + W].bitcast(U16),
                    op=ALU.add,
                )
                add_insts[i] = (a1, a2)

            # ---- PE: box-sum matmuls into PSUM pair tiles
            s_ps = {}
            for i in range(IMGS):
                for g in range(2):
                    s2 = ps2.tile([128, 2, W], F32, tag="s2", name=f"s{i}{g}")
                    s_ps[(i, g)] = s2
                    for j in range(2):
                        w = 2 * g + j
                        nc.tensor.matmul(
                            s2[:, j, :], band_sb[:, 0, :], v3_sb[i][:, w, 0:W],
                            start=True, stop=False)
                        nc.tensor.matmul(
                            s2[:, j, :], band_sb[:, 0, :], v_sb[i][:, w, 1:W + 1],
                            start=False, stop=True)
            sT = ps1.tile([128, W], F32)
            nc.tensor.matmul(sT[:], band_sb[:, 1, :], v3T[:, 0:W],
                             start=True, stop=False)
            nc.tensor.matmul(sT[:], band_sb[:, 1, :], vT[:, 1:W + 1],
                             start=False, stop=True)

            # ---- ACT pipeline: tail first (pkt lands first), then exp/ln
            # per image; image 3's ln split so its BMRs start earlier.
            # Order is pinned with an explicit dep chain (the Tile
            # scheduler otherwise batches by readiness).
            act_chain = []
            eyT = singles.tile([128, W], BF16)
            act_chain.append(nc.scalar.activation(
                eyT[:], pkt_sb[:, TP:PKC], ACTF.Exp))
            spyT = singles.tile([128, W], BF16)
            act_chain.append(nc.scalar.activation(
                spyT[:], eyT[:], ACTF.Ln, bias=1.0,
                accum_out=stats_sb[:, 5:6]))
            ey_sb = [None] * IMGS
            spy_sb = [None] * IMGS
            ln_insts = {}
            for i in range(IMGS):
                ey_sb[i] = eyp.tile([128, NWIN, W], BF16, tag="ey", name=f"ey{i}")
                act_chain.append(nc.scalar.activation(
                    ey_sb[i][:], pk_sb[i][:, :, TP:PKC], ACTF.Exp))
                spy_sb[i] = spyp.tile([128, NWIN, W], BF16, tag="spy",
                                      name=f"spy{i}")
                if i < 3:
                    ln = nc.scalar.activation(
                        spy_sb[i][:], ey_sb[i][:], ACTF.Ln, bias=1.0,
                        accum_out=stats_sb[:, i:i + 1])
                    ln_insts[(i, 0)] = ln_insts[(i, 1)] = ln
                    act_chain.append(ln)
                else:
                    ln_insts[(i, 0)] = nc.scalar.activation(
                        spy_sb[i][:, 0:2, :], ey_sb[i][:, 0:2, :], ACTF.Ln,
                        bias=1.0, accum_out=stats_sb[:, 3:4])
                    ln_insts[(i, 1)] = nc.scalar.activation(
                        spy_sb[i][:, 2:4, :], ey_sb[i][:, 2:4, :], ACTF.Ln,
                        bias=1.0, accum_out=stats_sb[:, 4:5])
                    act_chain.append(ln_insts[(i, 0)])
                    act_chain.append(ln_insts[(i, 1)])
            for a, b in zip(act_chain[1:], act_chain[:-1]):
                tile.add_dep_helper(a.ins, b.ins, sync=True,
                                    reason="pin ACT order")

            # ---- DVE phase 2: fused boundary-mask-mul-reduce; tail first
            scrT = scrp.tile([128, W], BF16, tag="scr", name="scrT")
            dve_chain.append(nc.vector._custom_dve(
                BMR, out=scrT[:], in0=sT[:], in1=spyT[:],
                s0=-12.5 * SC, s1=QTHR,
                accum_out=stats_sb[:, NA + 8:NA + 9]))
            for i in range(IMGS):
                dve_chain.append(add_insts[i][0])
                dve_chain.append(add_insts[i][1])
            for i in range(IMGS):
                for g in range(2):
                    scr = scrp.tile([128, 2, W], BF16, tag="scr",
                                    name=f"scr{i}{g}")
                    col = NA + 2 * i + g
                    dve_chain.append(nc.vector._custom_dve(
                        BMR, out=scr[:], in0=s_ps[(i, g)][:],
                        in1=spy_sb[i][:, 2 * g:2 * g + 2, :],
                        s0=-12.5 * SC, s1=QTHR,
                        accum_out=stats_sb[:, col:col + 1]))
            for a, b in zip(dve_chain[1:], dve_chain[:-1]):
                tile.add_dep_helper(a.ins, b.ins, sync=True,
                                    reason="pin DVE order")

            nc.gpsimd.dma_start(stats[:, :], stats_sb[:])

    nc.compile()
    nc.finalize()
    return nc


_NC = None


def _get_nc() -> bass.Bass:
    global _NC
    if _NC is None:
        _NC = _build_nc()
    return _NC


def _make_in_maps(pred: np.ndarray, target: np.ndarray) -> list[dict]:
    import ml_dtypes

    fp8 = ml_dtypes.float8_e4m3fn
    x8 = pred.reshape(B, H, W).astype(fp8)
    t_u8 = target.reshape(B, H, W).astype(np.uint8)
    ysig = (x8.view(np.uint8) ^ (t_u8 << 7))            # presigned, uint8
    junk = np.asarray(-240.0, dtype=fp8).view(np.uint8).item()   # 0xF7

    tpad = np.zeros((B, TP, TP), dtype=np.uint8)
    tpad[:, PAD:PAD + H, PAD:PAD + W] = t_u8

    # main windows: tpad rows 124w + p
    win_is = [0, 124, 248, 372]
    rows = np.asarray(win_is)[:, None] + np.arange(128)[None, :]  # [4, 128]
    twin = tpad[:, rows, :].transpose(0, 2, 1, 3)        # [B,128,4,516] u8

    ymain = np.full((B, 128, NWIN, W), junk, dtype=np.uint8)
    for g in range(NWIN):
        ymain[:, 2:126, g, :] = ysig[:, 124 * g:124 * g + 124, :]

    pkm = np.ascontiguousarray(
        np.concatenate([twin, ymain], axis=3)).view(fp8)  # [B,128,4,1028]

    # tail slab, per core: partition 20j+r = tpad row 496+r of image j;
    # y at partition 16j+k = presigned row 496+k of image j
    band_m = np.zeros((128, 128), dtype=np.float32)
    for m in range(2, 126):
        band_m[m - 2:m + 3, m] = 1.0
    band_t = np.zeros((128, 128), dtype=np.float32)
    for j in range(IMGS):
        for k in range(16):
            band_t[20 * j + k:20 * j + k + 5, 16 * j + k] = 1.0
    bands = np.stack([band_m, band_t], axis=1).astype(fp8)   # [128, 2, 128]

    in_maps = []
    for c in range(NCORES):
        sl = slice(c * IMGS, (c + 1) * IMGS)
        ttail = np.zeros((128, TP), dtype=np.uint8)
        ytail = np.full((128, W), junk, dtype=np.uint8)
        for j in range(IMGS):
            ttail[20 * j:20 * j + 20, :] = tpad[c * IMGS + j, 496:516, :]
            ytail[16 * j:16 * j + 16, :] = ysig[c * IMGS + j, 496:512, :]
        pkt = np.ascontiguousarray(
            np.concatenate([ttail, ytail], axis=1)).view(fp8)  # [128, 1028]
        in_maps.append(
            {
                "pkm": np.ascontiguousarray(pkm[sl]),
                "pkt": pkt,
                "bands": bands,
            }
        )
    return in_maps


def _finish(results: list[dict]) -> np.ndarray:
    total = 0.0
    for res in results:
        st = res["stats"].astype(np.float64)
        total += st[:, 0:NA].sum()
        total += 4.0 * st[:, NA:].sum()
    mean = total / float(B * H * W)
    return np.asarray(np.float32(mean))


def kernel(pred: np.ndarray, target: np.ndarray, **run_kwargs) -> np.ndarray:
    pred = np.asarray(pred)
    target = np.asarray(target)
    nc = _get_nc()
    in_maps = _make_in_maps(pred, target)
    out = run_bass_kernel_spmd(nc, in_maps, core_ids=list(range(NCORES)), **run_kwargs)
    res = _finish(out.results)
    kernel.last_run = out
    return res
